# revision 26
# baseline (speedup 1.0000x reference)
"""Trainium2 Bass kernel: single-head causal attention, SPMD over 8 NeuronCores.

Problem: x [4, 2048, 1024] f32; Wq/Wk/Wv [1024, 64]; bq/bk/bv [64].
  q,k,v = x@W + b ; out = softmax(causal(q k^T / 8)) @ v  -> [4, 2048, 64]

Sharding (uniform SPMD structure on every core):
  core c -> batch b = c//2 ; query chunks (cA, cB) = (c%2, 3-c%2), 512 rows
  each (pairing an early with a late chunk balances causal work).  Every core
  computes K/V for its batch's full 2048 rows.

Key layout: the k-axis is permuted PER CORE to chunk order
  [cA, 1-cA, 5-cB, cB], so the core's own query columns sit at the STATIC
  positions 0:512 and 1536:2048 of the K/V input.  With that permutation the
  24 (slot, k-tile) score tiles fall into three static classes:
    diag        A:0-3,  B:12-15  -- element-wise causal mask
    conditional A:4-7,  B:8-11   -- fully dead or fully allowed per core
    full        B:0-7            -- causally full for every core
  Masking is folded into the exp: diag tiles add a precomputed 0/+512 bias
  tile then exp(s*scale - 64); conditional tiles add a per-core 0/-400 bias
  column; dead tiles underflow to exactly 0 in fp16, so no mask multiplies
  and the 65th "ones" V row still accumulates the correct denominator.

  Projections produce Q^T/K^T/V^T [64, rows]; scores are computed transposed
  ([k_part, q_free]) so the weight matrix feeds the AV matmul as the moving
  operand; V is re-transposed through 16 small PE transposes.  Both slots
  accumulate AV in a single K=128 PSUM accumulator.  Slot-A attention and
  the V transposes are interleaved into the h1 projection stream so the PE
  never idles (keeps the tensor clock in its high p-state).

  Input DMAs are coalesced: xk is repacked host-side to [128, 16*1024] in
  consumption order, loaded by 8 large descriptors spread over the
  sync/scalar/vector hardware queues; tiny constants ride the gpsimd queue.

dtypes: fp16 SBUF operands, fp32 PSUM accumulation, fp32 epilogue + output.
"""

import os
import sys

import numpy as np

if "/opt/trn_rl_repo" not in sys.path:
    sys.path.insert(0, "/opt/trn_rl_repo")

B, S, D, H = 4, 2048, 1024, 64
CH = 512          # query chunk width
QR = 2 * CH       # query rows per core
NKT = S // 128    # 16 k-tiles of 128
SCALE = 1.0 / np.sqrt(H)
MBIG = 512.0      # diag mask additive bias (exactly representable, *SCALE=64)
CBIG = 400.0      # conditional (dead-tile) bias magnitude

_CACHE = {}


def _build_nc():
    import concourse.bacc as bacc
    import concourse.mybir as mybir
    import concourse.tile as tile

    DT = mybir.dt.float16
    F32 = mybir.dt.float32
    Exp = mybir.ActivationFunctionType.Exp
    Copy = mybir.ActivationFunctionType.Copy
    ge = mybir.AluOpType.is_ge
    mult = mybir.AluOpType.mult
    add = mybir.AluOpType.add

    nc = bacc.Bacc("TRN2", target_bir_lowering=False, debug=False, num_devices=8)

    # xk: k-permuted x^T, [128, 16*1024]; column block h*8+kt holds
    # dmodel-tile kt of k-half h (consumption order, so one coalesced
    # stream fills it front to back).
    xk = nc.dram_tensor("xk", [128, 16 * 1024], DT, kind="ExternalInput")
    wkv = nc.dram_tensor("wkv", [128, 8 * 128], DT, kind="ExternalInput")
    wq = nc.dram_tensor("wq", [128, 8 * H], DT, kind="ExternalInput")
    # constant blobs: cb32 = [bkv | bq | thrd(8) | thrb(8)],
    # cb16 = [qio(512) | idv(64) | id16(65)]
    cb32 = nc.dram_tensor("cb32", [128, 18], F32, kind="ExternalInput")
    cb16 = nc.dram_tensor("cb16", [128, CH + H + H + 1], DT,
                          kind="ExternalInput")
    out = nc.dram_tensor("out", [QR, H], F32, kind="ExternalOutput")

    def xcol(kt, h, sub=0, n=512):
        base = (h * 8 + kt) * 1024 + sub * 512
        return slice(base, base + n)

    with tile.TileContext(nc) as tc:
        with (
            tc.tile_pool(name="const", bufs=1) as cp,
            tc.tile_pool(name="work", bufs=8) as wp,
            tc.tile_pool(name="epi", bufs=4) as ep,
        ):
            # ---- coalesced input DMAs: first pieces split across both hw
            # queues for latency, then big consumption-ordered pieces ----
            wkv_sb = cp.tile([128, 8 * 128], DT, tag="wkv", name="wkv")
            xk_sb = cp.tile([128, 16 * 1024], DT, tag="xk", name="xk")
            wq_sb = cp.tile([128, 8 * H], DT, tag="wq", name="wq")
            cb32_sb = cp.tile([128, 18], F32, tag="cb32", name="cb32")
            cb16_sb = cp.tile([128, CH + 2 * H + 1], DT, tag="cb16",
                              name="cb16")

            # per-kt partition halves split across both hw queues: many
            # small DMAs in flight drive aggregate DGE bandwidth, and the
            # first tiles land with minimum latency
            nc.sync.dma_start(wkv_sb[0:64, :], wkv[0:64, :])
            nc.scalar.dma_start(wkv_sb[64:128, :], wkv[64:128, :])
            for kt in range(8):
                c = slice(kt * 1024, (kt + 1) * 1024)
                nc.sync.dma_start(xk_sb[0:64, c], xk[0:64, c])
                nc.scalar.dma_start(xk_sb[64:128, c], xk[64:128, c])
                if kt == 0:
                    nc.sync.dma_start(wq_sb[:], wq[:])
                    nc.scalar.dma_start(cb32_sb[:], cb32[:])
            # h1 tiles whole, interleaved across queues
            for kt in range(8):
                c = slice((8 + kt) * 1024, (9 + kt) * 1024)
                (nc.sync if kt % 2 == 0 else nc.scalar).dma_start(
                    xk_sb[:, c], xk[:, c])
                if kt == 0:
                    nc.sync.dma_start(cb16_sb[:], cb16[:])

            kvT_sb = cp.tile([128, S], DT, tag="kvT", name="kvT")  # 0:64 K^T, 64:128 V^T
            qT_sb = cp.tile([H, QR], DT, tag="qT", name="qT")      # A cols 0:512, B 512:1024
            v_sb = cp.tile([128, NKT * (H + 1)], DT, tag="v", name="v")
            mb_sb = cp.tile([128, 8 * CH], DT, tag="mb", name="mb")  # diag 0/1 mask

            # ---- projections in two 3-bank PSUM phase scopes ----
            def proj_group(h, kt, kv_ps, q_ps, first, last):
                for sub in range(2):
                    nc.tensor.matmul(
                        kv_ps[sub][:],
                        wkv_sb[:, kt * 128:(kt + 1) * 128],
                        xk_sb[:, xcol(kt, h, sub)],
                        start=first, stop=last,
                    )
                # q columns: slot A = sub 0 of h0, slot B = sub 1 of h1
                nc.tensor.matmul(
                    q_ps[:],
                    wq_sb[:, kt * H:(kt + 1) * H],
                    xk_sb[:, xcol(kt, h, sub=h)],
                    start=first, stop=last,
                )

            def proj_epilogue(h, kv_ps, q_ps):
                for sub in range(2):
                    nb = 2 * h + sub
                    nc.vector.tensor_scalar(
                        kvT_sb[:, nb * 512:(nb + 1) * 512], kv_ps[sub][:],
                        cb32_sb[:, 0:1], None, add)
                nc.vector.tensor_scalar(
                    qT_sb[:, h * 512:(h + 1) * 512], q_ps[:],
                    cb32_sb[0:H, 1:2], None, add)

            # ---- phase h0, consumption order matching DMA arrival ----
            H0_ORDER = (0, 1, 2, 3, 4, 5, 6, 7)
            pp0 = tc.alloc_tile_pool(name="proj_ps0", bufs=1, space="PSUM")
            kv_ps0 = [pp0.tile([128, 512], F32, tag=f"kvps0{s}",
                               name=f"kvps0{s}") for s in range(2)]
            q_ps0 = pp0.tile([H, 512], F32, tag="qps0", name="qps0")
            for i, kt in enumerate(H0_ORDER):
                proj_group(0, kt, kv_ps0, q_ps0, first=(i == 0), last=(i == 7))
            proj_epilogue(0, kv_ps0, q_ps0)
            pp0.release()

            sp = tc.alloc_tile_pool(name="score_ps", bufs=4, space="PSUM")
            avpA = tc.alloc_tile_pool(name="avA_ps", bufs=1, space="PSUM")
            av_a = avpA.tile([H + 1, 512], F32, tag="avA", name="avA")

            # diag 0/1 fp16 masks (A tiles 0..3 -> cols 0..3, B tiles
            # 12..15 -> cols 4..7)
            for j in range(8):
                nc.vector.tensor_scalar(
                    mb_sb[:, j * CH:(j + 1) * CH], cb16_sb[:, 0:CH],
                    cb32_sb[:, 2 + j:3 + j], None, ge)

            def vtr(kt):
                t = sp.tile([128, H], DT, tag="score", name=f"vtr{kt}")
                nc.tensor.transpose(
                    t[:], kvT_sb[H:128, kt * 128:(kt + 1) * 128],
                    cb16_sb[H:128, CH:CH + H], tile_position=(H, 0))
                nc.vector.tensor_copy(
                    v_sb[:, kt * (H + 1):kt * (H + 1) + H], t[:])

            nc.vector.memset(v_sb[:], 1.0)
            for kt in range(8):
                vtr(kt)

            # ---- slot helpers ----
            def score(slot, kt):
                s_ps = sp.tile([128, 512], F32, tag="score", name=f"s{slot}{kt}")
                nc.tensor.matmul(
                    s_ps[:],
                    kvT_sb[0:H, kt * 128:(kt + 1) * 128],
                    qT_sb[:, slot * 512:(slot + 1) * 512],
                    start=True, stop=True, tile_position=(0, 0),
                )
                return s_ps

            def wexp(slot, kt, s_ps):
                # tile class: diag (exp then 0/1 mask multiply), cond
                # (per-core bias column folded into exp), full (plain exp)
                w_sb = wp.tile([128, 512], DT, tag="wexp", name="wexp")
                if slot == 0 and kt < 4:
                    j = kt                         # A diag
                elif slot == 1 and kt >= 12:
                    j = kt - 8                     # B diag
                else:
                    j = None
                if j is not None:
                    nc.scalar.activation(w_sb[:], s_ps[:], Exp,
                                         scale=float(SCALE))
                    wm_sb = wp.tile([128, 512], DT, tag="wm", name="wm")
                    nc.vector.tensor_tensor(
                        wm_sb[:], w_sb[:], mb_sb[:, j * CH:(j + 1) * CH],
                        mult)
                    return wm_sb
                if (slot == 0 and kt >= 4) or (slot == 1 and 8 <= kt < 12):
                    nc.scalar.activation(w_sb[:], s_ps[:], Exp,
                                         bias=cb32_sb[:, 6 + kt:7 + kt],
                                         scale=float(SCALE))
                else:                              # B full (kt < 8)
                    nc.scalar.activation(w_sb[:], s_ps[:], Exp,
                                         scale=float(SCALE))
                return w_sb

            def av(acc, kt, w_sb, first, last):
                vs = slice(kt * (H + 1), (kt + 1) * (H + 1))
                nc.tensor.matmul(
                    acc[:], v_sb[:, vs], w_sb[:],
                    start=first, stop=last,
                )

            # ---- phase h1 interleaved with slot A attention ----
            pp1 = tc.alloc_tile_pool(name="proj_ps1", bufs=1, space="PSUM")
            kv_ps1 = [pp1.tile([128, 512], F32, tag=f"kvps1{s}",
                               name=f"kvps1{s}") for s in range(2)]
            q_ps1 = pp1.tile([H, 512], F32, tag="qps1", name="qps1")

            a_s = {}
            a_w = {}

            def asc_pair(p):
                for kt in (2 * p, 2 * p + 1):
                    a_s[kt] = score(0, kt)
                    a_w[kt] = wexp(0, kt, a_s[kt])

            def ava_pair(p):
                for kt in (2 * p, 2 * p + 1):
                    av(av_a, kt, a_w[kt], first=(kt == 0), last=(kt == 7))

            for kt in range(8):
                proj_group(1, kt, kv_ps1, q_ps1, first=(kt == 0),
                           last=(kt == 7))
                # interleave slot-A work behind the projection groups
                if kt == 0:
                    asc_pair(0)
                elif kt == 1:
                    asc_pair(1)
                elif kt == 2:
                    ava_pair(0)
                elif kt == 3:
                    asc_pair(2)
                elif kt == 4:
                    ava_pair(1)
                elif kt == 5:
                    asc_pair(3)
                elif kt == 6:
                    ava_pair(2)
                else:
                    ava_pair(3)
            proj_epilogue(1, kv_ps1, q_ps1)
            pp1.release()

            avpB = tc.alloc_tile_pool(name="avB_ps", bufs=1, space="PSUM")
            av_b = avpB.tile([H + 1, 512], F32, tag="avB", name="avB")
            sp2 = tc.alloc_tile_pool(name="otr_ps", bufs=2, space="PSUM")

            def epilogue(slot, acc):
                oav_sb = ep.tile([H + 1, 512], DT, tag="oav16", name="oav")
                nc.vector.tensor_copy(oav_sb[:], acc[:])
                for j in range(4):
                    tr_ps = sp2.tile([128, H + 1], DT, tag="otr", name="otr")
                    nc.tensor.transpose(
                        tr_ps[:],
                        oav_sb[:, j * 128:(j + 1) * 128],
                        cb16_sb[0:H + 1, CH + H:CH + 2 * H + 1],
                    )
                    r_sb = ep.tile([128, 1], F32, tag="recip", name="recip")
                    nc.vector.reciprocal(r_sb[:], tr_ps[:, H:H + 1])
                    o_sb = ep.tile([128, H], F32, tag="osb", name="osb")
                    nc.vector.tensor_scalar_mul(o_sb[:], tr_ps[:, 0:H], r_sb[:])
                    row = slot * CH + j * 128
                    # sync/scalar only: gpsimd stores would hold up its
                    # end-of-kernel queue drain
                    (nc.sync if j % 2 == 0 else nc.scalar).dma_start(
                        out[row:row + 128, :], o_sb[:])

            # ---- slot B: V transposes for h1 tiles, masked-first scores,
            # single-accumulator AV (3-deep score prefetch); slot-A
            # epilogue interleaves ----
            kts = list(range(8, 16)) + list(range(8))
            b_w = {}
            for kt in (8, 9, 10, 11):
                vtr(kt)
            for kt in (8, 9):
                b_w[kt] = wexp(1, kt, score(1, kt))
            for kt in (12, 13, 14, 15):
                vtr(kt)
            b_w[10] = wexp(1, 10, score(1, 10))

            for i, kt in enumerate(kts):
                if kt not in b_w:
                    b_w[kt] = wexp(1, kt, score(1, kt))
                av(av_b, kt, b_w.pop(kt), first=(i == 0), last=(i == NKT - 1))
                nxt = kts[i + 3] if i + 3 < NKT else None
                if nxt is not None and nxt not in b_w:
                    b_w[nxt] = wexp(1, nxt, score(1, nxt))
                if i == 1:
                    epilogue(0, av_a)
            epilogue(1, av_b)

            for pool in (sp2, avpB, avpA, sp):
                pool.release()

    nc.compile()
    return nc


def _host_inputs(x, Wq, bq, Wk, bk, Wv, bv):
    """Build the 8 per-core input maps (all SBUF-layout, fp16/f32)."""
    f16 = np.float16
    Wkv = np.concatenate([Wk, Wv], axis=1)          # [D, 128]
    # wkv[p, kt*128+j] = Wkv[kt*128+p, j]
    wkv_np = np.ascontiguousarray(
        Wkv.reshape(8, 128, 128).transpose(1, 0, 2).reshape(128, 8 * 128)
    ).astype(f16)
    wq_np = np.zeros((128, 8 * H), dtype=f16)
    for kt in range(8):
        wq_np[:, kt * H:(kt + 1) * H] = Wq[kt * 128:(kt + 1) * 128, :]
    # cb16 = [qio(512) | idv(64) | id16(65)]
    cb16_np = np.zeros((128, CH + 2 * H + 1), dtype=f16)
    cb16_np[:, 0:CH] = np.arange(CH, dtype=f16)
    cb16_np[:, CH:CH + H] = np.concatenate([np.eye(H), np.eye(H)], axis=0)
    cb16_np[0:H + 1, CH + H:] = np.eye(H + 1)

    in_maps = []
    for c in range(8):
        b = c // 2
        cA, cB = c % 2, 3 - c % 2
        perm = (cA, 1 - cA, 5 - cB, cB)        # chunk order along k
        xTp = np.concatenate(
            [x[b, p * CH:(p + 1) * CH].T for p in perm], axis=1)  # [D, S]
        xTp = xTp.astype(f16)
        # xk[p, (h*8+kt)*1024 + c] = xTp[kt*128+p, h*1024+c]
        xk_np = np.ascontiguousarray(
            xTp.reshape(8, 128, 2, 1024).transpose(1, 2, 0, 3)
            .reshape(128, 16 * 1024))
        # k_global of permuted position p: perm[p//512]*512 + p%512
        pos = np.arange(S)
        kg = np.array(perm)[pos // CH] * CH + pos % CH
        p = np.arange(128)
        # cb32 = [bkv | bq(pad) | thrd(8) | thrb(8, pre-scaled)]
        cb32_np = np.zeros((128, 18), dtype=np.float32)
        cb32_np[:, 0] = np.concatenate([bk, bv])
        cb32_np[0:H, 1] = bq
        for j in range(4):                      # A diag tiles 0..3
            cb32_np[:, 2 + j] = kg[j * 128 + p] - cA * CH
        for j in range(4):                      # B diag tiles 12..15
            cb32_np[:, 6 + j] = kg[(12 + j) * 128 + p] - cB * CH
        cb32_np[:, 10:14] = (-CBIG * SCALE) if (1 - cA) > cA else 0.0
        cb32_np[:, 14:18] = (-CBIG * SCALE) if (5 - cB) > cB else 0.0
        in_maps.append({
            "xk": xk_np, "wkv": wkv_np, "wq": wq_np,
            "cb32": cb32_np, "cb16": cb16_np,
        })
    return in_maps


def _gather(results, dtype):
    y = np.zeros((B, S, H), dtype=dtype)
    for c in range(8):
        b = c // 2
        cA, cB = c % 2, 3 - c % 2
        o = results[c]["out"]
        y[b, cA * CH:(cA + 1) * CH] = o[:CH]
        y[b, cB * CH:(cB + 1) * CH] = o[CH:]
    return y


def get_nc():
    if "nc" not in _CACHE:
        _CACHE["nc"] = _build_nc()
    return _CACHE["nc"]


def kernel(x, Wq, bq, Wk, bk, Wv, bv, _trace=False, _trace_kwargs=None):
    from concourse.bass_utils import run_bass_kernel_spmd

    x = np.asarray(x, dtype=np.float32)
    Wq, bq = np.asarray(Wq, np.float32), np.asarray(bq, np.float32)
    Wk, bk = np.asarray(Wk, np.float32), np.asarray(bk, np.float32)
    Wv, bv = np.asarray(Wv, np.float32), np.asarray(bv, np.float32)

    nc = get_nc()
    in_maps = _host_inputs(x, Wq, bq, Wk, bk, Wv, bv)
    res = run_bass_kernel_spmd(
        nc, in_maps, core_ids=list(range(8)),
        trace=_trace, **(_trace_kwargs or {}))
    _CACHE["last_result"] = res
    return _gather(res.results, x.dtype)


# revision 36
# speedup vs baseline: 1.1590x; 1.1590x over previous
"""Trainium2 Bass kernel: single-head causal attention, SPMD over 8 NeuronCores.

Problem: x [4, 2048, 1024] f32; Wq/Wk/Wv [1024, 64]; bq/bk/bv [64].
  q,k,v = x@W + b ; out = softmax(causal(q k^T / 8)) @ v  -> [4, 2048, 64]

Sharding (uniform SPMD structure on every core):
  core c -> batch b = c//2 ; query chunks (cA, cB) = (c%2, 3-c%2), 512 rows
  each (pairing an early with a late chunk balances causal work).  Every core
  computes K/V for its batch's full 2048 rows.

Key layout: the k-axis is permuted PER CORE to chunk order
  [cA, 1-cA, 5-cB, cB], so the core's own query columns sit at the STATIC
  positions 0:512 and 1536:2048 of the K/V input.  With that permutation the
  24 (slot, k-tile) score tiles fall into three static classes:
    diag        A:0-3,  B:12-15  -- element-wise causal mask
    conditional A:4-7,  B:8-11   -- fully dead or fully allowed per core
    full        B:0-7            -- causally full for every core
  Masking is folded into the exp: diag tiles add a precomputed 0/+512 bias
  tile then exp(s*scale - 64); conditional tiles add a per-core 0/-400 bias
  column; dead tiles underflow to exactly 0 in fp16, so no mask multiplies
  and the 65th "ones" V row still accumulates the correct denominator.

  Projections produce Q^T/K^T/V^T [64, rows]; scores are computed transposed
  ([k_part, q_free]) so the weight matrix feeds the AV matmul as the moving
  operand; V is re-transposed through 16 small PE transposes.  Both slots
  accumulate AV in a single K=128 PSUM accumulator.  Slot-A attention and
  the V transposes are interleaved into the h1 projection stream so the PE
  never idles (keeps the tensor clock in its high p-state).

  Input DMAs are coalesced: xk is repacked host-side to [128, 16*1024] in
  consumption order, loaded by 8 large descriptors spread over the
  sync/scalar/vector hardware queues; tiny constants ride the gpsimd queue.

dtypes: fp16 SBUF operands, fp32 PSUM accumulation, fp32 epilogue + output.
"""

import os
import sys

import numpy as np

if "/opt/trn_rl_repo" not in sys.path:
    sys.path.insert(0, "/opt/trn_rl_repo")

B, S, D, H = 4, 2048, 1024, 64
CH = 512          # query chunk width
QR = 2 * CH       # query rows per core
NKT = S // 128    # 16 k-tiles of 128
SCALE = 1.0 / np.sqrt(H)
MBIG = 512.0      # diag mask additive bias (exactly representable, *SCALE=64)
CBIG = 400.0      # conditional (dead-tile) bias magnitude

_CACHE = {}


def _build_nc():
    import concourse.bacc as bacc
    import concourse.mybir as mybir
    import concourse.tile as tile

    DT = mybir.dt.float16
    F32 = mybir.dt.float32
    Exp = mybir.ActivationFunctionType.Exp
    Copy = mybir.ActivationFunctionType.Copy
    ge = mybir.AluOpType.is_ge
    mult = mybir.AluOpType.mult
    add = mybir.AluOpType.add

    nc = bacc.Bacc("TRN2", target_bir_lowering=False, debug=False, num_devices=8)

    # xk: k-permuted x^T, [128, 16*1024]; column block h*8+kt holds
    # dmodel-tile kt of k-half h (consumption order, so one coalesced
    # stream fills it front to back).
    xk = nc.dram_tensor("xk", [128, 16 * 1024], DT, kind="ExternalInput")
    wkv = nc.dram_tensor("wkv", [128, 8 * 128], DT, kind="ExternalInput")
    wq = nc.dram_tensor("wq", [128, 8 * H], DT, kind="ExternalInput")
    # constant blobs: cb32 = [bkv | bq | thrd(8) | thrb(8)],
    # cb16 = [qio(512) | idv(64) | id16(65)]
    cb32 = nc.dram_tensor("cb32", [128, 18], F32, kind="ExternalInput")
    cb16 = nc.dram_tensor("cb16", [128, CH + H + H + 1], DT,
                          kind="ExternalInput")
    out = nc.dram_tensor("out", [QR, H], F32, kind="ExternalOutput")

    def xcol(kt, h, sub=0, n=512):
        # slab layout: [h0 tiles (kt*1024+sub*512) | qB block (8192+kt*512)
        #               | h1kv block (12288+kt*512)]
        if h == 0:
            base = kt * 1024 + sub * 512
        elif sub == 1:
            base = 8192 + kt * 512      # k-positions 1536:2048 (q-B cols)
        else:
            base = 12288 + kt * 512     # k-positions 1024:1536
        return slice(base, base + n)

    with tile.TileContext(nc) as tc:
        with (
            tc.tile_pool(name="const", bufs=1) as cp,
            tc.tile_pool(name="work", bufs=8) as wp,
            tc.tile_pool(name="epi", bufs=4) as ep,
        ):
            # ---- coalesced input DMAs: first pieces split across both hw
            # queues for latency, then big consumption-ordered pieces ----
            wkv_sb = cp.tile([128, 8 * 128], DT, tag="wkv", name="wkv")
            xk_sb = cp.tile([128, 16 * 1024], DT, tag="xk", name="xk")
            wq_sb = cp.tile([128, 8 * H], DT, tag="wq", name="wq")
            cb32_sb = cp.tile([128, 18], F32, tag="cb32", name="cb32")
            cb16_sb = cp.tile([128, CH + 2 * H + 1], DT, tag="cb16",
                              name="cb16")

            # medium pieces, h0-priority: first four DMAs per queue carry
            # all of h0 (distinct completion sems -> max in-flight depth);
            # small constants slot in where queue-FIFO order permits
            def dmas(q, pieces):
                for lo, hi in pieces:
                    q.dma_start(xk_sb[:, lo:hi], xk[:, lo:hi])

            nc.sync.dma_start(wkv_sb[0:64, :], wkv[0:64, :])
            nc.scalar.dma_start(wkv_sb[64:128, :], wkv[64:128, :])
            nc.sync.dma_start(xk_sb[0:64, 0:1024], xk[0:64, 0:1024])
            nc.scalar.dma_start(xk_sb[64:128, 0:1024], xk[64:128, 0:1024])
            nc.sync.dma_start(wq_sb[:], wq[:])
            nc.scalar.dma_start(cb32_sb[:], cb32[:])
            dmas(nc.sync, [(1024, 2048), (2048, 4096)])      # kt1, kt2-3
            dmas(nc.scalar, [(4096, 6144), (6144, 8192)])    # kt4-5, kt6-7
            nc.sync.dma_start(cb16_sb[:], cb16[:])
            dmas(nc.sync, [(8192, 10240)])                   # qB kt0-3
            dmas(nc.scalar, [(10240, 12288)])                # qB kt4-7
            dmas(nc.scalar, [(12288, 14336)])                # h1kv kt0-3
            dmas(nc.sync, [(14336, 16384)])                  # h1kv kt4-7

            kvT_sb = cp.tile([128, S], DT, tag="kvT", name="kvT")  # 0:64 K^T, 64:128 V^T
            qT_sb = cp.tile([H, QR], DT, tag="qT", name="qT")      # A cols 0:512, B 512:1024
            v_sb = cp.tile([128, NKT * (H + 1)], DT, tag="v", name="v")
            mb_sb = cp.tile([128, 8 * CH], DT, tag="mb", name="mb")  # diag 0/1 mask

            # ---- projections in two 3-bank PSUM phase scopes ----
            def proj_group(h, kt, kv_ps, q_ps, first, last):
                for sub in range(2):
                    nc.tensor.matmul(
                        kv_ps[sub][:],
                        wkv_sb[:, kt * 128:(kt + 1) * 128],
                        xk_sb[:, xcol(kt, h, sub)],
                        start=first, stop=last,
                    )
                if q_ps is not None:  # slot-A q rides h0 (sub 0 columns)
                    nc.tensor.matmul(
                        q_ps[:],
                        wq_sb[:, kt * H:(kt + 1) * H],
                        xk_sb[:, xcol(kt, h, sub=0)],
                        start=first, stop=last,
                    )

            def kv_epilogue(h, kv_ps):
                for sub in range(2):
                    nb = 2 * h + sub
                    nc.vector.tensor_scalar(
                        kvT_sb[:, nb * 512:(nb + 1) * 512], kv_ps[sub][:],
                        cb32_sb[:, 0:1], None, add)

            def q_epilogue(slot, q_ps):
                nc.vector.tensor_scalar(
                    qT_sb[:, slot * 512:(slot + 1) * 512], q_ps[:],
                    cb32_sb[0:H, 1:2], None, add)

            # ---- phase h0 (+ q-B projection filling the h0->h1 DMA gap),
            # consumption order matching DMA arrival ----
            H0_ORDER = (0, 1, 4, 5, 2, 3, 6, 7)
            pp0 = tc.alloc_tile_pool(name="proj_ps0", bufs=1, space="PSUM")
            ppq = tc.alloc_tile_pool(name="projq_ps", bufs=1, space="PSUM")
            kv_ps0 = [pp0.tile([128, 512], F32, tag=f"kvps0{s}",
                               name=f"kvps0{s}") for s in range(2)]
            q_ps0 = pp0.tile([H, 512], F32, tag="qps0", name="qps0")
            qb_ps = ppq.tile([H, 512], F32, tag="qpsb", name="qpsb")
            for i, kt in enumerate(H0_ORDER):
                proj_group(0, kt, kv_ps0, q_ps0, first=(i == 0), last=(i == 7))
            for kt in range(8):
                nc.tensor.matmul(
                    qb_ps[:],
                    wq_sb[:, kt * H:(kt + 1) * H],
                    xk_sb[:, xcol(kt, 1, sub=1)],
                    start=(kt == 0), stop=(kt == 7),
                )
            kv_epilogue(0, kv_ps0)
            q_epilogue(0, q_ps0)
            q_epilogue(1, qb_ps)
            ppq.release()
            pp0.release()

            sp = tc.alloc_tile_pool(name="score_ps", bufs=3, space="PSUM")
            avpA = tc.alloc_tile_pool(name="avA_ps", bufs=1, space="PSUM")
            av_a = avpA.tile([H + 1, 512], F32, tag="avA", name="avA")
            avpB = tc.alloc_tile_pool(name="avB_ps", bufs=1, space="PSUM")
            av_b = avpB.tile([H + 1, 512], F32, tag="avB", name="avB")

            # diag 0/1 fp16 masks (A tiles 0..3 -> cols 0..3, B tiles
            # 12..15 -> cols 4..7)
            for j in range(8):
                nc.vector.tensor_scalar(
                    mb_sb[:, j * CH:(j + 1) * CH], cb16_sb[:, 0:CH],
                    cb32_sb[:, 2 + j:3 + j], None, ge)

            def vtr(kt):
                t = sp.tile([128, H], DT, tag="score", name=f"vtr{kt}")
                nc.tensor.transpose(
                    t[:], kvT_sb[H:128, kt * 128:(kt + 1) * 128],
                    cb16_sb[H:128, CH:CH + H], tile_position=(H, 0))
                nc.vector.tensor_copy(
                    v_sb[:, kt * (H + 1):kt * (H + 1) + H], t[:])

            nc.vector.memset(v_sb[:], 1.0)

            # ---- slot helpers ----
            def score(slot, kt):
                s_ps = sp.tile([128, 512], F32, tag="score", name=f"s{slot}{kt}")
                nc.tensor.matmul(
                    s_ps[:],
                    kvT_sb[0:H, kt * 128:(kt + 1) * 128],
                    qT_sb[:, slot * 512:(slot + 1) * 512],
                    start=True, stop=True, tile_position=(0, 0),
                )
                return s_ps

            def wexp(slot, kt, s_ps):
                # tile class: diag (exp then 0/1 mask multiply), cond
                # (per-core bias column folded into exp), full (plain exp)
                w_sb = wp.tile([128, 512], DT, tag="wexp", name="wexp")
                if slot == 0 and kt < 4:
                    j = kt                         # A diag
                elif slot == 1 and kt >= 12:
                    j = kt - 8                     # B diag
                else:
                    j = None
                if j is not None:
                    nc.scalar.activation(w_sb[:], s_ps[:], Exp,
                                         scale=float(SCALE))
                    wm_sb = wp.tile([128, 512], DT, tag="wm", name="wm")
                    nc.vector.tensor_tensor(
                        wm_sb[:], w_sb[:], mb_sb[:, j * CH:(j + 1) * CH],
                        mult)
                    return wm_sb
                if (slot == 0 and kt >= 4) or (slot == 1 and 8 <= kt < 12):
                    nc.scalar.activation(w_sb[:], s_ps[:], Exp,
                                         bias=cb32_sb[:, 6 + kt:7 + kt],
                                         scale=float(SCALE))
                else:                              # B full (kt < 8)
                    nc.scalar.activation(w_sb[:], s_ps[:], Exp,
                                         scale=float(SCALE))
                return w_sb

            def av(acc, kt, w_sb, first, last):
                vs = slice(kt * (H + 1), (kt + 1) * (H + 1))
                nc.tensor.matmul(
                    acc[:], v_sb[:, vs], w_sb[:],
                    start=first, stop=last,
                )

            # ---- pre-h1 fill: V transposes + all slot-A scores + first
            # slot-B-full scores run while the h1kv stream arrives ----
            a_w = {}
            b_w = {}
            avb_n = [0]

            def avb(kt, w_sb):
                i = avb_n[0]
                avb_n[0] += 1
                av(av_b, kt, w_sb, first=(i == 0), last=(i == NKT - 1))

            for kt in (0, 1, 2, 3):
                vtr(kt)
            for kt in (0, 1):
                a_w[kt] = wexp(0, kt, score(0, kt))
            for kt in (4, 5, 6, 7):
                vtr(kt)
            for kt in (2, 3):
                a_w[kt] = wexp(0, kt, score(0, kt))
            b_w[0] = wexp(1, 0, score(1, 0))
            for kt in (4, 5):
                a_w[kt] = wexp(0, kt, score(0, kt))
            b_w[1] = wexp(1, 1, score(1, 1))
            for kt in (6, 7):
                a_w[kt] = wexp(0, kt, score(0, kt))

            # ---- phase h1 (kv only) interleaved with A/B-full AV ----
            pp1 = tc.alloc_tile_pool(name="proj_ps1", bufs=1, space="PSUM")
            kv_ps1 = [pp1.tile([128, 512], F32, tag=f"kvps1{s}",
                               name=f"kvps1{s}") for s in range(2)]

            H1_PLAN = {
                0: [("ava", 0), ("bsc", 2)],
                1: [("ava", 1), ("bsc", 3)],
                2: [("ava", 2), ("ava", 3), ("bsc", 4)],
                3: [("avb", 0), ("bsc", 5)],
                4: [("ava", 4), ("avb", 1), ("bsc", 6)],
                5: [("ava", 5), ("avb", 2), ("bsc", 7)],
                6: [("ava", 6), ("avb", 3)],
                7: [("ava", 7), ("avb", 4)],
            }
            for kt in range(8):
                proj_group(1, kt, kv_ps1, None, first=(kt == 0),
                           last=(kt == 7))
                for op, t in H1_PLAN[kt]:
                    if op == "ava":
                        av(av_a, t, a_w.pop(t), first=(t == 0), last=(t == 7))
                    elif op == "avb":
                        avb(t, b_w.pop(t))
                    else:
                        b_w[t] = wexp(1, t, score(1, t))
            kv_epilogue(1, kv_ps1)
            pp1.release()

            sp2 = tc.alloc_tile_pool(name="otr_ps", bufs=2, space="PSUM")

            def epilogue(slot, acc):
                oav_sb = ep.tile([H + 1, 512], DT, tag="oav16", name="oav")
                if slot == 0:
                    nc.vector.tensor_copy(oav_sb[:], acc[:])
                else:
                    # split so the first transpose starts 3 copies earlier
                    for j in range(4):
                        js = slice(j * 128, (j + 1) * 128)
                        nc.vector.tensor_copy(oav_sb[:, js], acc[:, js])
                for j in range(4):
                    tr_ps = sp2.tile([128, H + 1], DT, tag="otr", name="otr")
                    nc.tensor.transpose(
                        tr_ps[:],
                        oav_sb[:, j * 128:(j + 1) * 128],
                        cb16_sb[0:H + 1, CH + H:CH + 2 * H + 1],
                    )
                    r_sb = ep.tile([128, 1], F32, tag="recip", name="recip")
                    nc.vector.reciprocal(r_sb[:], tr_ps[:, H:H + 1])
                    o_sb = ep.tile([128, H], F32, tag="osb", name="osb")
                    nc.vector.tensor_scalar_mul(o_sb[:], tr_ps[:, 0:H], r_sb[:])
                    row = slot * CH + j * 128
                    # sync/scalar only: gpsimd stores would hold up its
                    # end-of-kernel queue drain
                    (nc.sync if j % 2 == 0 else nc.scalar).dma_start(
                        out[row:row + 128, :], o_sb[:])

            # ---- B tail: V transposes for h1 tiles, masked tiles
            # (diag first), trailing B-full AVs; slot-A epilogue early ----
            tail = (12, 13, 14, 15, 8, 9, 10, 11)
            for kt in (8, 9, 10, 11):
                vtr(kt)
            epilogue(0, av_a)
            for kt in (12, 13, 14, 15):
                vtr(kt)
            for kt in (12, 13):
                b_w[kt] = wexp(1, kt, score(1, kt))
            avb(5, b_w.pop(5))
            b_w[14] = wexp(1, 14, score(1, 14))
            avb(6, b_w.pop(6))
            b_w[15] = wexp(1, 15, score(1, 15))
            avb(7, b_w.pop(7))
            for i, kt in enumerate(tail):
                if kt not in b_w:
                    b_w[kt] = wexp(1, kt, score(1, kt))
                avb(kt, b_w.pop(kt))
                nxt = tail[i + 3] if i + 3 < 8 else None
                if nxt is not None and nxt not in b_w:
                    b_w[nxt] = wexp(1, nxt, score(1, nxt))
            epilogue(1, av_b)

            for pool in (sp2, avpB, avpA, sp):
                pool.release()

    nc.compile()
    return nc


def _host_inputs(x, Wq, bq, Wk, bk, Wv, bv):
    """Build the 8 per-core input maps (all SBUF-layout, fp16/f32)."""
    f16 = np.float16
    Wkv = np.concatenate([Wk, Wv], axis=1)          # [D, 128]
    # wkv[p, kt*128+j] = Wkv[kt*128+p, j]
    wkv_np = np.ascontiguousarray(
        Wkv.reshape(8, 128, 128).transpose(1, 0, 2).reshape(128, 8 * 128)
    ).astype(f16)
    wq_np = np.zeros((128, 8 * H), dtype=f16)
    for kt in range(8):
        wq_np[:, kt * H:(kt + 1) * H] = Wq[kt * 128:(kt + 1) * 128, :]
    # cb16 = [qio(512) | idv(64) | id16(65)]
    cb16_np = np.zeros((128, CH + 2 * H + 1), dtype=f16)
    cb16_np[:, 0:CH] = np.arange(CH, dtype=f16)
    cb16_np[:, CH:CH + H] = np.concatenate([np.eye(H), np.eye(H)], axis=0)
    cb16_np[0:H + 1, CH + H:] = np.eye(H + 1)

    in_maps = []
    for c in range(8):
        b = c // 2
        cA, cB = c % 2, 3 - c % 2
        perm = (cA, 1 - cA, 5 - cB, cB)        # chunk order along k
        xTp = np.concatenate(
            [x[b, p * CH:(p + 1) * CH].T for p in perm], axis=1)  # [D, S]
        xTp = xTp.astype(f16)
        # slab: [h0: kt*1024+c (pos 0:1024) | qB: 8192+kt*512 (pos
        # 1536:2048) | h1kv: 12288+kt*512 (pos 1024:1536)]
        base = xTp.reshape(8, 128, 2048)            # [kt, p, pos]
        xk_np = np.zeros((128, 16 * 1024), dtype=f16)
        xk_np[:, 0:8192] = base[:, :, 0:1024].transpose(1, 0, 2).reshape(
            128, 8192)
        xk_np[:, 8192:12288] = base[:, :, 1536:2048].transpose(1, 0, 2
            ).reshape(128, 4096)
        xk_np[:, 12288:16384] = base[:, :, 1024:1536].transpose(1, 0, 2
            ).reshape(128, 4096)
        # k_global of permuted position p: perm[p//512]*512 + p%512
        pos = np.arange(S)
        kg = np.array(perm)[pos // CH] * CH + pos % CH
        p = np.arange(128)
        # cb32 = [bkv | bq(pad) | thrd(8) | thrb(8, pre-scaled)]
        cb32_np = np.zeros((128, 18), dtype=np.float32)
        cb32_np[:, 0] = np.concatenate([bk, bv])
        cb32_np[0:H, 1] = bq
        for j in range(4):                      # A diag tiles 0..3
            cb32_np[:, 2 + j] = kg[j * 128 + p] - cA * CH
        for j in range(4):                      # B diag tiles 12..15
            cb32_np[:, 6 + j] = kg[(12 + j) * 128 + p] - cB * CH
        cb32_np[:, 10:14] = (-CBIG * SCALE) if (1 - cA) > cA else 0.0
        cb32_np[:, 14:18] = (-CBIG * SCALE) if (5 - cB) > cB else 0.0
        in_maps.append({
            "xk": xk_np, "wkv": wkv_np, "wq": wq_np,
            "cb32": cb32_np, "cb16": cb16_np,
        })
    return in_maps


def _gather(results, dtype):
    y = np.zeros((B, S, H), dtype=dtype)
    for c in range(8):
        b = c // 2
        cA, cB = c % 2, 3 - c % 2
        o = results[c]["out"]
        y[b, cA * CH:(cA + 1) * CH] = o[:CH]
        y[b, cB * CH:(cB + 1) * CH] = o[CH:]
    return y


def get_nc():
    if "nc" not in _CACHE:
        _CACHE["nc"] = _build_nc()
    return _CACHE["nc"]


def kernel(x, Wq, bq, Wk, bk, Wv, bv, _trace=False, _trace_kwargs=None):
    from concourse.bass_utils import run_bass_kernel_spmd

    x = np.asarray(x, dtype=np.float32)
    Wq, bq = np.asarray(Wq, np.float32), np.asarray(bq, np.float32)
    Wk, bk = np.asarray(Wk, np.float32), np.asarray(bk, np.float32)
    Wv, bv = np.asarray(Wv, np.float32), np.asarray(bv, np.float32)

    nc = get_nc()
    in_maps = _host_inputs(x, Wq, bq, Wk, bk, Wv, bv)
    res = run_bass_kernel_spmd(
        nc, in_maps, core_ids=list(range(8)),
        trace=_trace, **(_trace_kwargs or {}))
    _CACHE["last_result"] = res
    return _gather(res.results, x.dtype)
